# revision 5
# baseline (speedup 1.0000x reference)
"""GCNConvNet on 8 Trainium2 NeuronCores (Bass/Tile SPMD kernel).

Design (measured on HW via repeat-slope timing; ~1.8x faster per conv layer
than the fp32 predecessor):
  - Whole z-table path in bf16: z rows, AllGather, edge gathers, S matrix,
    weights.  PSUM accumulation stays f32.  Halves collective + gather +
    S-stream traffic and quadruples PE matmul rate (bf16 1 cyc/row vs fp32 4).
  - Self-loops are regular gather edges (their src row is the node's own z
    row) — the separate Sself/zown path is gone.
  - Nodes split into lo/hi halves per core; the per-layer AllGather is TWO
    collectives (lo, hi).  Aggregation runs in two passes: pass A consumes
    chunks 0,1 (lo-half z of cores 0-3 / 4-7) and overlaps AG_hi; pass B
    consumes chunks 2,3, finishes the psum, applies bias/act, and emits
    next-layer z (issuing AG_lo(next) right after the lo tiles are emitted).
    Pass A's partial sums park in SBUF f32 accumulators (PSUM has only 8
    banks; 26 live tiles don't fit).
  - Gathers batched 2 tiles per dma_gather call to amortize SWDGE
    descriptor-generation overhead on the Pool/Q7 engine.

Geometry: G=25 dsts/group, TGP=20 groups/tile -> DT=500 dst/tile,
NGROUPS=520/core, T=26 tiles (13 lo + 13 hi), NPCP=13000 padded ids/core,
chunk rows = 4 cores x 6500 = 26000 <= int16 idx limit.
"""

import math
import numpy as np

import concourse.bass as bass
import concourse.bacc as bacc
import concourse.tile as tile
import concourse.mybir as mybir
from concourse.bass_utils import run_bass_kernel_spmd

F32 = mybir.dt.float32
BF16 = mybir.dt.bfloat16
I16 = mybir.dt.int16
AF = mybir.ActivationFunctionType

CORES = 8
CHUNKS = 4
P = 128


class Cfg:
    def __init__(self, n_nodes):
        assert n_nodes % (4 * CORES) == 0
        self.N = n_nodes
        self.NPC = n_nodes // CORES            # real nodes per core (12500)
        self.G = 25                            # dsts per group
        self.TGP = 20                          # groups per tile
        self.DT = self.G * self.TGP            # 500 dsts per tile
        self.NGROUPS = 520                     # per core (260 per half)
        self.NGH = self.NGROUPS // 2
        self.NPCH = self.NGH * self.G          # padded ids per half (6500)
        self.NPCP = 2 * self.NPCH              # padded ids per core (13000)
        self.NP = self.NPCP * CORES
        self.T = self.NGROUPS // self.TGP      # 26 tiles
        self.TH = self.T // 2                  # 13 per half
        self.CR = 4 * self.NPCH                # rows per gather chunk (26000)
        assert self.CR <= 32767
        assert self.NPC <= self.NPCP
        assert self.DT <= 512
        # one gather call per (tile, chunk); slots compact-packed at
        # 16-granularity per group (plan computed in preprocess)
        # S columns: contiguous region of 2*TH*DT per chunk
        self.s_total = CHUNKS * self.NPCP


# ---------------------------------------------------------------------------
# host preprocessing
# ---------------------------------------------------------------------------

def _group_greedy(dvec, n_groups, gsize, cap=128):
    """Assign len(dvec) items into n_groups of <=gsize items each, keeping
    every per-chunk (4-dim) load <= cap.  dvec: [n,4] int."""
    n = dvec.shape[0]
    order = np.argsort(-dvec.sum(axis=1), kind="stable")
    loads = np.zeros((n_groups, CHUNKS), np.int64)
    sizes = np.zeros(n_groups, np.int64)
    group_of = np.empty(n, np.int64)
    for i in order:
        cand = np.max(loads + dvec[i], axis=1).astype(np.float64)
        cand[sizes >= gsize] = np.inf
        lim = sizes.min() + 2
        cand[sizes >= lim] = np.inf
        g = int(np.argmin(cand))
        group_of[i] = g
        loads[g] += dvec[i]
        sizes[g] += 1
    stall = 0
    for _ in range(80000):
        gbad, cbad = np.unravel_index(np.argmax(loads), loads.shape)
        worst = loads[gbad, cbad]
        if worst <= cap or stall > 60:
            break
        members = np.flatnonzero(group_of == gbad)
        others = np.flatnonzero(group_of != gbad)
        do = dvec[others]
        improved = False
        for i in members[np.argsort(-dvec[members, cbad])[:4]]:
            di = dvec[i]
            base_g = loads[gbad] - di
            cand_g = np.max(base_g + do, axis=1)
            base_o = loads[group_of[others]] - do
            cand_o = np.max(base_o + di, axis=1)
            score = np.maximum(cand_g, cand_o)
            j = others[int(np.argmin(score))]
            if score.min() < worst:
                gj = group_of[j]
                loads[gbad] += dvec[j] - di
                loads[gj] += di - dvec[j]
                group_of[i] = gj
                group_of[j] = gbad
                improved = True
                break
        stall = 0 if improved else stall + 1
    return group_of, loads


def _snake(n_items, n_ways):
    pattern = np.concatenate([np.arange(n_ways), np.arange(n_ways)[::-1]])
    reps = math.ceil(n_items / (2 * n_ways))
    return np.tile(pattern, reps)[:n_items]


def preprocess(x, edge_index, cfg: Cfg):
    N = cfg.N
    src_r = np.asarray(edge_index[0], np.int64)
    dst_r = np.asarray(edge_index[1], np.int64)

    deg = np.bincount(dst_r, minlength=N).astype(np.float64) + 1.0
    dinv = 1.0 / np.sqrt(deg)

    # self loops included as regular edges
    loop = np.arange(N, dtype=np.int64)
    srcs = np.concatenate([src_r, loop])
    dsts = np.concatenate([dst_r, loop])
    norms = (dinv[srcs] * dinv[dsts]).astype(np.float32)

    # core assignment: snake over degree-sorted nodes
    order = np.argsort(-deg, kind="stable")
    core_of = np.empty(N, np.int64)
    core_of[order] = _snake(N, CORES)
    counts = np.bincount(core_of, minlength=CORES)
    assert (counts == cfg.NPC).all(), counts

    # half assignment: snake within each core over its degree-sorted nodes
    half_of = np.empty(N, np.int64)
    hpat = np.array([0, 1, 1, 0])
    for k in range(CORES):
        nodes_k = order[core_of[order] == k]
        half_of[nodes_k] = hpat[np.arange(len(nodes_k)) % 4]

    chunk_of = half_of * 2 + (core_of >= 4)

    # per-dst chunk-degree vectors (self loops included)
    dvec = np.zeros((N, CHUNKS), np.int64)
    np.add.at(dvec, (dsts, chunk_of[srcs]), 1)

    # within-(core,half) grouping
    local_of = np.empty(N, np.int64)
    for k in range(CORES):
        for h in range(2):
            nodes = np.flatnonzero((core_of == k) & (half_of == h))
            gof, loads = _group_greedy(dvec[nodes], cfg.NGH, cfg.G)
            assert loads.max() <= 128, (
                f"core {k} half {h}: group chunk load {loads.max()} > 128"
            )
            # relabel groups by descending max-chunk-load so the same
            # (tile, slot) position has similar loads on every core (the
            # compiled gather plan is shared and uses the per-position max)
            order_g = np.argsort(-loads.max(axis=1), kind="stable")
            relabel = np.empty(cfg.NGH, np.int64)
            relabel[order_g] = np.arange(cfg.NGH)
            gof = relabel[gof]
            o = np.argsort(gof, kind="stable")
            gsorted = gof[o]
            first = np.r_[0, np.flatnonzero(np.diff(gsorted)) + 1]
            fo = np.zeros(cfg.NGH, np.int64)
            fo[gsorted[first]] = first
            rank = np.arange(len(o)) - fo[gsorted]
            local_of[nodes[o]] = h * cfg.NPCH + gsorted * cfg.G + rank
    new_of = core_of * cfg.NPCP + local_of

    # z-table row within gather chunk
    row_in_chunk = (core_of % 4) * cfg.NPCH + (local_of % cfg.NPCH)

    # edge bucket packing
    e_core = core_of[dsts]
    e_loc = local_of[dsts]
    e_g = e_loc // cfg.G
    e_pos = e_loc % cfg.G
    e_t = e_g // cfg.TGP
    e_gt = e_g % cfg.TGP
    e_c = chunk_of[srcs]
    e_lsrc = row_in_chunk[srcs]
    assert e_lsrc.max() < cfg.CR

    bucket = (e_core * cfg.NGROUPS + e_g) * CHUNKS + e_c
    so = np.argsort(bucket, kind="stable")
    sb = bucket[so]
    starts = np.r_[0, np.flatnonzero(np.diff(sb)) + 1]
    sizes = np.diff(np.r_[starts, len(sb)])
    assert sizes.max() <= 128, f"bucket overflow: {sizes.max()}"
    start_of = np.zeros(CORES * cfg.NGROUPS * CHUNKS, np.int64)
    start_of[sb[starts]] = starts
    rank = np.arange(len(sb)) - start_of[sb]

    core_s = e_core[so]
    t_s = e_t[so]
    gt_s = e_gt[so]
    c_s = e_c[so]
    pos_s = e_pos[so]
    lsrc_s = e_lsrc[so]
    norm_s = norms[so]

    # ---- compact slot packing ----
    # loads[core, t, gt, c] = bucket sizes (identical plan needed across
    # cores for a single compiled program: use per-(t,gt,c) MAX over cores)
    loads = np.zeros((CORES, cfg.T, cfg.TGP, CHUNKS), np.int64)
    np.add.at(loads, (e_core, e_t, e_gt, e_c), 1)
    lmax = loads.max(axis=0)                      # [T, TGP, CHUNKS]
    assert lmax.max() <= 128
    # PE matmul tile bases are limited to {0,32,64}; sub-128 packing would
    # fragment matmuls and cross-core plan sharing eats most of the gain —
    # keep full 128-slot buckets.
    cap16 = np.full_like(lmax, 128)
    # group slot offsets within each (t, c) window
    off_g = np.zeros((cfg.T, cfg.TGP, CHUNKS), np.int64)
    ncall = np.zeros((cfg.T, CHUNKS), np.int64)   # num_idxs per call
    for t in range(cfg.T):
        for c in range(CHUNKS):
            o = np.concatenate([[0], np.cumsum(cap16[t, :, c])])
            off_g[t, :, c] = o[:-1]
            ncall[t, c] = o[-1]
    # idx window offsets per (c, t): c-major then t (gather issue order)
    win_off = np.zeros((CHUNKS, cfg.T), np.int64)
    off = 0
    plan_windows = {}
    for c in range(CHUNKS):
        for t in range(cfg.T):
            win_off[c, t] = off
            plan_windows[(c, t)] = (off, int(ncall[t, c]) // 16,
                                    int(ncall[t, c]))
            off += int(ncall[t, c]) // 16
    idx_total = off

    # idx array; unused in-window slots point at random valid rows (S=0)
    rng_pad = np.random.default_rng(12345)
    idx_all = rng_pad.integers(0, cfg.CR, (CORES, 16, idx_total),
                               dtype=np.int16)
    slot = off_g[t_s, gt_s, c_s] + rank
    icol = win_off[c_s, t_s] + slot // 16
    irow = slot % 16
    idx_all[core_s, irow, icol] = lsrc_s.astype(np.int16)

    # mm pieces per (t, c, g): split at 128-slot block boundaries
    pieces = {}
    for t in range(cfg.T):
        for c in range(CHUNKS):
            pl = []
            for g in range(cfg.TGP):
                o, L = int(off_g[t, g, c]), int(cap16[t, g, c])
                while L > 0:
                    b, p0 = o // 128, o % 128
                    take = min(L, 128 - p0)
                    pl.append((g, b, p0, take))
                    o += take
                    L -= take
            pieces[(t, c)] = pl

    plan = dict(windows=plan_windows, pieces=pieces, idx_total=idx_total,
                max_blocks=int(((ncall + 127) // 128).max()))

    # S array [128, s_total]; row = slot % 128;
    # col = c*NPCP + h*NPCH + (t_in_half*TGP+gt)*G + pos
    s_all = np.zeros((CORES, P, cfg.s_total), np.float32)
    h_s = t_s // cfg.TH
    tin_s = t_s % cfg.TH
    scol = (c_s * cfg.NPCP + h_s * cfg.NPCH
            + (tin_s * cfg.TGP + gt_s) * cfg.G + pos_s)
    s_all[core_s, slot % 128, scol] = norm_s

    # x shards, feature-major, zero at holes
    xT_all = np.zeros((CORES, P, cfg.NPCP), np.float32)
    xx = np.asarray(x, np.float32)
    n_core = new_of // cfg.NPCP
    n_local = new_of % cfg.NPCP
    xT_all[n_core, :, n_local] = xx

    return dict(idx_all=idx_all, s_all=s_all, xT_all=xT_all,
                newpos_of_old=new_of.copy(), plan=plan)


# ---------------------------------------------------------------------------
# bass kernel
# ---------------------------------------------------------------------------

def build_nc(cfg: Cfg, plan, repeat=1):
    nc = bacc.Bacc("TRN2", target_bir_lowering=False, debug=False,
                   num_devices=CORES, num_swdge_queues=4)

    idx_total = plan["idx_total"]
    max_blocks = plan["max_blocks"]
    xT = nc.dram_tensor("xT", [P, cfg.NPCP], BF16, kind="ExternalInput")
    idxd = nc.dram_tensor("idx", [16, idx_total], I16, kind="ExternalInput")
    sd = nc.dram_tensor("S", [P, cfg.s_total], BF16, kind="ExternalInput")
    wd = nc.dram_tensor("W", [P, 4 * P], BF16, kind="ExternalInput")
    bd = nc.dram_tensor("B", [P, 4], F32, kind="ExternalInput")
    lw1d = nc.dram_tensor("lw1", [P, 64], BF16, kind="ExternalInput")
    lb1d = nc.dram_tensor("lb1", [64, 1], F32, kind="ExternalInput")
    lw2d = nc.dram_tensor("lw2", [64, 1], BF16, kind="ExternalInput")
    lb2d = nc.dram_tensor("lb2", [1, 1], F32, kind="ExternalInput")
    outd = nc.dram_tensor("out", [cfg.NPCP, 1], F32, kind="ExternalOutput")

    zbuf = [nc.dram_tensor(f"zbuf{h}", [cfg.NPCH, P], BF16) for h in range(2)]
    zfull = [nc.dram_tensor(f"zfull{h}", [CORES * cfg.NPCH, P], BF16,
                            addr_space="Shared") for h in range(2)]

    TH, DT, TGP, G = cfg.TH, cfg.DT, cfg.TGP, cfg.G
    n_layers = 4 * repeat

    with tile.TileContext(nc) as tc:
        with tc.tile_pool(name="const", bufs=1) as cp, \
             tc.tile_pool(name="sb", bufs=3) as sbp, \
             tc.tile_pool(name="spool", bufs=2) as sp, \
             tc.tile_pool(name="mpool", bufs=8) as mp, \
             tc.tile_pool(name="accp", bufs=cfg.T) as acp, \
             tc.tile_pool(name="psagg", bufs=4, space="PSUM") as pp_agg, \
             tc.tile_pool(name="psz", bufs=2, space="PSUM") as pp_z, \
             tc.tile_pool(name="pshead", bufs=1, space="PSUM") as pp_head:

            w_sb = cp.tile([P, 4 * P], BF16)
            nc.sync.dma_start(w_sb[:], wd[:, :])
            b_sb = cp.tile([P, 4], F32)
            nc.sync.dma_start(b_sb[:], bd[:, :])
            lw1_sb = cp.tile([P, 64], BF16)
            nc.sync.dma_start(lw1_sb[:], lw1d[:, :])
            lb1_sb = cp.tile([64, 1], F32)
            nc.sync.dma_start(lb1_sb[:], lb1d[:, :])
            lw2_sb = cp.tile([64, 1], BF16)
            nc.sync.dma_start(lw2_sb[:], lw2d[:, :])
            lb2_sb = cp.tile([1, 1], F32)
            nc.sync.dma_start(lb2_sb[:], lb2d[:, :])
            idx_sb = cp.tile([P, idx_total], I16)
            for q in range(8):
                nc.sync.dma_start(idx_sb[16 * q:16 * (q + 1), :], idxd[:, :])

            def emit_z(h_tile, layer, t):
                """z rows for tile t (local rows t*DT..+DT) of `layer`."""
                half = t // TH
                r0 = (t % TH) * DT
                for b in range(math.ceil(DT / P)):
                    s0 = b * P
                    sl = min(P, DT - s0)
                    zp = pp_z.tile([P, P], F32, tag="zp",
                                   name=f"zp{layer}_{t}_{b}")
                    nc.tensor.matmul(
                        zp[0:sl, :],
                        lhsT=h_tile[:, s0:s0 + sl],
                        rhs=w_sb[:, layer * P:(layer + 1) * P],
                        start=True, stop=True)
                    zs = sbp.tile([P, P], BF16, tag="zs",
                                  name=f"zs{layer}_{t}_{b}")
                    nc.vector.tensor_copy(zs[0:sl, :], zp[0:sl, :])
                    nc.sync.dma_start(
                        zbuf[half][r0 + s0:r0 + s0 + sl, :], zs[0:sl, :])

            def emit_head(h_tile, t):
                r0 = t * DT
                hp = pp_head.tile([64, DT], F32, tag="hp", name=f"hp{t}")
                nc.tensor.matmul(hp[:, 0:DT], lhsT=lw1_sb[:],
                                 rhs=h_tile[:, 0:DT], start=True, stop=True)
                ha = sbp.tile([64, DT], BF16, tag="ha", name=f"ha{t}")
                nc.scalar.activation(ha[:, 0:DT], hp[:, 0:DT], AF.Relu,
                                     bias=lb1_sb[:])
                op = pp_head.tile([1, DT], F32, tag="op", name=f"op{t}")
                nc.tensor.matmul(op[:, 0:DT], lhsT=lw2_sb[:], rhs=ha[0:64, 0:DT],
                                 start=True, stop=True)
                ob = sbp.tile([1, DT], F32, tag="ob", name=f"ob{t}")
                nc.scalar.activation(ob[:, 0:DT], op[:, 0:DT], AF.Sigmoid,
                                     bias=lb2_sb[:])
                nc.sync.dma_start(
                    outd[r0:r0 + DT, :].rearrange("a b -> b a"), ob[:, 0:DT])

            def allgather(half):
                nc.gpsimd.collective_compute(
                    "AllGather", mybir.AluOpType.bypass,
                    replica_groups=[list(range(CORES))],
                    ins=[zbuf[half].ap()], outs=[zfull[half].ap()])

            rr = [0]

            def gather_tile(c, t, li):
                off, ncols, nidx = plan["windows"][(c, t)]
                blocks = (nidx + 127) // 128
                m = mp.tile([P, max_blocks * P], BF16, tag="m",
                            name=f"m{li}_{c}_{t}")
                m3 = m[:, 0:blocks * P].rearrange("p (b e) -> p b e", e=P)
                q = rr[0] % 4
                rr[0] += 1
                nc.gpsimd.dma_gather(
                    m3, zfull[c // 2][(c % 2) * cfg.CR:(c % 2 + 1) * cfg.CR, :],
                    idx_sb[:, off:off + ncols],
                    nidx, nidx, P, single_packet=False, queue_num=q)
                return m

            # ---- layer 0 z from x ----
            for t in range(cfg.T):
                xt = sbp.tile([P, DT], BF16, tag="xt", name=f"xt{t}")
                nc.sync.dma_start(xt[:, 0:DT], xT[:, t * DT:(t + 1) * DT])
                emit_z(xt, 0, t)
                if t == TH - 1:
                    allgather(0)
            allgather(1)

            # ---- conv layers ----
            for li in range(n_layers):
                layer = li % 4
                last = li == n_layers - 1
                accs = {}
                for passno in range(2):
                    cpair = (0, 1) if passno == 0 else (2, 3)
                    s_tiles = {}
                    for c in cpair:
                        st = sp.tile([P, cfg.NPCP], BF16, tag="s",
                                     name=f"s{li}_{c}")
                        nc.sync.dma_start(
                            st[:, :],
                            sd[:, c * cfg.NPCP:(c + 1) * cfg.NPCP])
                        s_tiles[c] = st
                    for t in range(cfg.T):
                        mA = gather_tile(cpair[0], t, li)
                        mB = gather_tile(cpair[1], t, li)
                        h = t // TH
                        tin = t % TH
                        ps = pp_agg.tile([P, DT], F32, tag="agg",
                                         name=f"agg{li}_{passno}_{t}")
                        col0 = h * cfg.NPCH + tin * TGP * G
                        # pieces: (g, block, p0, rows); start on the first
                        # piece of each g (chunk A), stop on its last (B)
                        pcA = plan["pieces"][(t, cpair[0])]
                        pcB = plan["pieces"][(t, cpair[1])]
                        firstA = {}
                        lastB = {}
                        for idx_p, (g, b, p0, rows) in enumerate(pcA):
                            firstA.setdefault(g, idx_p)
                        for idx_p, (g, b, p0, rows) in enumerate(pcB):
                            lastB[g] = idx_p
                        for idx_p, (g, b, p0, rows) in enumerate(pcA):
                            nc.tensor.matmul(
                                ps[:, g * G:(g + 1) * G],
                                lhsT=mA[p0:p0 + rows, b * P:(b + 1) * P],
                                rhs=s_tiles[cpair[0]][
                                    p0:p0 + rows,
                                    col0 + g * G:col0 + (g + 1) * G],
                                start=(firstA[g] == idx_p), stop=False)
                        for idx_p, (g, b, p0, rows) in enumerate(pcB):
                            nc.tensor.matmul(
                                ps[:, g * G:(g + 1) * G],
                                lhsT=mB[p0:p0 + rows, b * P:(b + 1) * P],
                                rhs=s_tiles[cpair[1]][
                                    p0:p0 + rows,
                                    col0 + g * G:col0 + (g + 1) * G],
                                start=False, stop=(lastB[g] == idx_p))
                        if passno == 0:
                            acc = acp.tile([P, DT], F32, tag="acc",
                                           name=f"acc{li}_{t}")
                            nc.vector.tensor_copy(acc[:, :], ps[:, 0:DT])
                            accs[t] = acc
                        else:
                            hst = sbp.tile([P, DT], BF16, tag="hst",
                                           name=f"hst{li}_{t}")
                            nc.vector.tensor_tensor(
                                hst[:, :], ps[:, 0:DT], accs[t][:, :],
                                mybir.AluOpType.add)
                            htl = sbp.tile([P, DT], BF16, tag="h",
                                           name=f"h{li}_{t}")
                            nc.scalar.activation(
                                htl[:, :], hst[:, :],
                                AF.Relu if layer < 3 else AF.Identity,
                                bias=b_sb[:, layer:layer + 1])
                            if last:
                                emit_head(htl, t)
                            else:
                                emit_z(htl, (layer + 1) % 4, t)
                                if t == TH - 1:
                                    allgather(0)
                if not last:
                    allgather(1)

    nc.compile()
    return nc


# ---------------------------------------------------------------------------
# entry point
# ---------------------------------------------------------------------------

_CACHE = {}


def _get_nc(cfg: Cfg, plan, repeat=1):
    key = (cfg.N, repeat)
    if key not in _CACHE:
        _CACHE[key] = build_nc(cfg, plan, repeat=repeat)
    return _CACHE[key]


def make_in_maps(pre, inputs, cfg: Cfg):
    import ml_dtypes
    bf = ml_dtypes.bfloat16
    W = np.concatenate([np.asarray(inputs[f"w{i}"], np.float32)
                        for i in range(4)], axis=1).astype(bf)
    B = np.stack([np.asarray(inputs[f"b{i}"], np.float32)
                  for i in range(4)], axis=1)
    in_maps = []
    for k in range(CORES):
        in_maps.append({
            "xT": pre["xT_all"][k].astype(bf),
            "idx": pre["idx_all"][k],
            "S": pre["s_all"][k].astype(bf),
            "W": W,
            "B": B,
            "lw1": np.asarray(inputs["lw1"], np.float32).astype(bf),
            "lb1": np.asarray(inputs["lb1"], np.float32).reshape(64, 1),
            "lw2": np.asarray(inputs["lw2"], np.float32).astype(bf),
            "lb2": np.asarray(inputs["lb2"], np.float32).reshape(1, 1),
        })
    return in_maps


def run(x, edge_index, inputs, cfg: Cfg, repeat=1):
    pre = preprocess(x, edge_index, cfg)
    in_maps = make_in_maps(pre, inputs, cfg)
    nc = _get_nc(cfg, pre["plan"], repeat)
    res = run_bass_kernel_spmd(nc, in_maps, core_ids=list(range(CORES)))
    out_new = np.concatenate([res.results[k]["out"] for k in range(CORES)],
                             axis=0)
    out = out_new[pre["newpos_of_old"]]
    return out, res


def kernel(x, edge_index, batch, w0, b0, w1, b1, w2, b2, w3, b3,
           lw1, lb1, lw2, lb2):
    x = np.asarray(x, np.float32)
    cfg = Cfg(x.shape[0])
    inputs = dict(w0=w0, b0=b0, w1=w1, b1=b1, w2=w2, b2=b2, w3=w3, b3=b3,
                  lw1=lw1, lb1=lb1, lw2=lw2, lb2=lb2)
    out, _ = run(x, edge_index, inputs, cfg)
    return out
